# revision 37
# baseline (speedup 1.0000x reference)
"""Trainium2 Bass kernel for nn_MessagePassingGNN (8-core SPMD).

Strategy:
  - Sort edges (with self-loops) by target node; shard TARGET NODES across
    the 8 cores (6250 each) so each core owns a contiguous edge range and
    the segment-sum aggregation is core-local (no all-reduce).
  - Per layer, each core gathers source-node features from a replicated
    bf16 feature table in DRAM via dma_gather(transpose=True), which yields
    feature-major tiles that feed the message-MLP matmuls directly (no
    on-chip transposes). Target-side gathers read a core-local shard table
    so they never wait on the collective.
  - The scatter-mean aggregation runs on the tensor engine: a scaled one-hot
    matrix P[e, n] = (tgt_rel[e] == n) / count[tgt_e] is built by one fused
    DVE tensor_scalar per 128-edge tile, then agg += m3_tile.T @ P_tile
    accumulates in PSUM per 128-target-node block.
  - GRU update is node-sharded; updated shard features are AllGather'd into
    every core's table for the next layer. The decoder runs on the local
    shard; the host concatenates the 8 shards.

Host->device staging is minimized: gather indices are staged compactly
([16, W] int16, replicated to the 128-partition wrap layout on device),
tgt_rel is int8, cinv is f16, and all bf16 weights are packed into one
blob that is sharded 1/8th per core and AllGather'd on device.

All matmuls are bf16 with fp32 PSUM accumulation; GRU elementwise math is
fp32. Host-measured end-to-end L2 relative error vs fp32 reference ~1e-2.
"""

import math

import numpy as np
import ml_dtypes

# Problem constants (hardcoded per harness contract).
N, IN_DIM, D, H, E, LAYERS = 50000, 16, 128, 256, 800000, 3
NCORES = 8
SH = N // NCORES            # 6250 nodes per shard
NB = (SH + 127) // 128      # 49 blocks of 128 target nodes
SHP = NB * 128              # 6272 padded shard width
SPLIT = 32768               # int16 index split for the gather table
BF16 = ml_dtypes.bfloat16

_PROGRAM_CACHE = {}
_RUN_KWARGS = {}       # test harness may set {"trace": True}
_LAST_RESULTS = None   # BassKernelResults of the most recent run

# Packed bf16 weight blob layout: (name, [P, K]) in order. Per-layer
# weights appear once per layer (l-major). Host packing and device
# unpacking both walk this table.
_WBF_LAYOUT = (
    [("wenc", [IN_DIM, 128])]
    + [
        (f"{nm}{l}", shp)
        for l in range(LAYERS)
        for nm, shp in [
            ("w1t", [128, H]), ("w1s", [128, H]),
            ("w2a", [128, H]), ("w2b", [128, H]),
            ("w3a", [128, D]), ("w3b", [128, D]),
            ("wih", [128, 3 * D]), ("whh", [128, 3 * D]),
        ]
    ]
    + [("wd1", [128, H]), ("wd2a", [128, H]), ("wd2b", [128, H]),
       ("wd3a", [128, 1]), ("wd3b", [128, 1])]
)
# f32 bias blob layout.
_BF32_LAYOUT = (
    [("benc", [128, 1])]
    + [
        (f"{nm}{l}", shp)
        for l in range(LAYERS)
        for nm, shp in [
            ("b1", [128, 2]), ("b2", [128, 2]), ("brz", [128, 2]),
            ("bgin", [128, 1]), ("bghn", [128, 1]),
        ]
    ]
    + [("bd1", [128, 2]), ("bd2", [128, 2])]
)
_WBF_TOTAL = sum(p * k for _, (p, k) in _WBF_LAYOUT)
_WBF_SH = -(-_WBF_TOTAL // NCORES)           # per-core slice (padded)
_BF32_TOTAL = sum(p * k for _, (p, k) in _BF32_LAYOUT)


# ----------------------------------------------------------------------------
# Host-side preprocessing
# ----------------------------------------------------------------------------

def _wrap_idx(idx_i16):
    """Compact dma_gather index layout: index i at [i % 16, i // 16].
    The 8x partition-group replication happens on device."""
    n = idx_i16.shape[0]
    return np.ascontiguousarray(idx_i16.reshape(n // 16, 16).T)


def _prep(x, edge_index):
    loops = np.arange(N, dtype=np.int64)
    src = np.concatenate([np.asarray(edge_index[0]), loops])
    tgt = np.concatenate([np.asarray(edge_index[1]), loops])
    order = np.argsort(tgt, kind="stable")
    src_s = src[order].astype(np.int32)
    tgt_s = tgt[order].astype(np.int32)
    counts = np.zeros(N, np.float32)
    np.add.at(counts, tgt_s, 1.0)
    cinv_n = (1.0 / counts).astype(np.float32)

    node_starts = np.searchsorted(tgt_s, np.arange(N + 1))
    lows = np.zeros((NCORES, NB), np.int64)
    highs = np.zeros((NCORES, NB), np.int64)
    rng = {}
    for c in range(NCORES):
        for b in range(NB):
            lo_node = c * SH + b * 128
            hi_node = min(c * SH + SH, lo_node + 128)
            e0, e1 = node_starts[lo_node], node_starts[hi_node]
            nl = int((src_s[e0:e1] < SPLIT).sum())
            lows[c, b] = nl
            highs[c, b] = (e1 - e0) - nl
            rng[(c, b)] = (e0, e1)
    Bl = [int(max(1, math.ceil(lows[:, b].max() / 128))) for b in range(NB)]
    Bh = [int(max(1, math.ceil(highs[:, b].max() / 128))) for b in range(NB)]

    meta = {"Bl": Bl, "Bh": Bh}
    ntiles = sum(Bl) + sum(Bh)
    nslots = ntiles * 128

    per_core = []
    for c in range(NCORES):
        idx_src = np.zeros(nslots, np.int16)
        tgt_rel = np.full(nslots, -1, np.int8)
        off = 0
        for b in range(NB):
            e0, e1 = rng[(c, b)]
            s, t = src_s[e0:e1], tgt_s[e0:e1]
            lo = s < SPLIT
            for mask, cap, base in ((lo, Bl[b], 0), (~lo, Bh[b], SPLIT)):
                sh_, th_ = s[mask], t[mask]
                n = sh_.shape[0]
                idx_src[off:off + n] = (sh_ - base).astype(np.int16)
                tgt_rel[off:off + n] = (th_ - (c * SH + b * 128)).astype(np.int8)
                off += cap * 128
        assert off == nslots

        src_cols = []
        off = 0
        for b in range(NB):
            wl, wh = Bl[b] * 128, Bh[b] * 128
            src_cols.append(_wrap_idx(idx_src[off:off + wl]))
            src_cols.append(_wrap_idx(idx_src[off + wl:off + wl + wh]))
            off += wl + wh

        xs = np.zeros((IN_DIM, SHP), np.float32)
        xs[:, :SH] = np.asarray(x[c * SH:(c + 1) * SH]).T
        cn = np.zeros((1, SHP), np.float32)
        cn[0, :SH] = cinv_n[c * SH:(c + 1) * SH]
        per_core.append({
            "x_sh_t": xs.astype(BF16),
            "idx_src": np.concatenate(src_cols, axis=1),
            "tgt_rel": tgt_rel.reshape(ntiles, 128).T.copy(),
            "cinv_n": cn,
        })
    return meta, per_core


def _prep_weights(inp):
    f32 = np.float32
    bf = lambda a: np.ascontiguousarray(np.asarray(a, f32)).astype(BF16)
    w = {}
    w["wenc"] = bf(inp["enc_W"])
    w["benc"] = np.asarray(inp["enc_b"], f32).reshape(128, 1)
    for l in range(LAYERS):
        w[f"w1t{l}"] = bf(inp["msg_W1"][l, :D, :])
        w[f"w1s{l}"] = bf(inp["msg_W1"][l, D:, :])
        w[f"w2a{l}"] = bf(inp["msg_W2"][l, :128, :])
        w[f"w2b{l}"] = bf(inp["msg_W2"][l, 128:, :])
        w[f"w3a{l}"] = bf(inp["msg_W3"][l, :128, :])
        w[f"w3b{l}"] = bf(inp["msg_W3"][l, 128:, :])
        w[f"wih{l}"] = bf(inp["gru_Wih"][l])
        w[f"whh{l}"] = bf(inp["gru_Whh"][l])
        w[f"b1{l}"] = np.asarray(inp["msg_b1"][l], f32).reshape(2, 128).T
        w[f"b2{l}"] = np.asarray(inp["msg_b2"][l], f32).reshape(2, 128).T
        bgi = (np.asarray(inp["msg_b3"][l], f32)
               @ np.asarray(inp["gru_Wih"][l], f32)
               + np.asarray(inp["gru_bih"][l], f32))
        bhh = np.asarray(inp["gru_bhh"][l], f32)
        w[f"brz{l}"] = (bgi[:2 * D] + bhh[:2 * D]).reshape(2, 128).T
        w[f"bgin{l}"] = bgi[2 * D:].reshape(128, 1)
        w[f"bghn{l}"] = bhh[2 * D:].reshape(128, 1)
    w["wd1"] = bf(inp["dec_W1"])
    w["wd2a"] = bf(inp["dec_W2"][:128, :])
    w["wd2b"] = bf(inp["dec_W2"][128:, :])
    w["wd3a"] = bf(inp["dec_W3"][:128, :])
    w["wd3b"] = bf(inp["dec_W3"][128:, :])
    w["bd1"] = np.asarray(inp["dec_b1"], f32).reshape(2, 128).T
    w["bd2"] = np.asarray(inp["dec_b2"], f32).reshape(2, 128).T

    wbf = np.zeros(_WBF_SH * NCORES, BF16)
    off = 0
    for nm, (p, k) in _WBF_LAYOUT:
        a = np.ascontiguousarray(w[nm])
        assert a.shape == (p, k) and a.dtype == BF16, (nm, a.shape, a.dtype)
        wbf[off:off + p * k] = a.ravel()
        off += p * k
    assert off == _WBF_TOTAL
    bf32 = np.zeros(_BF32_TOTAL, f32)
    off = 0
    for nm, (p, k) in _BF32_LAYOUT:
        a = np.ascontiguousarray(np.asarray(w[nm], f32))
        assert a.shape == (p, k), (nm, a.shape)
        bf32[off:off + p * k] = a.ravel()
        off += p * k
    assert off == _BF32_TOTAL
    return {"wbf": wbf, "bf32": bf32}


# ----------------------------------------------------------------------------
# Bass program
# ----------------------------------------------------------------------------

def _build_program(meta, debug=False, repeat=1):
    import concourse.bacc as bacc
    import concourse.mybir as mybir
    import concourse.tile as tile
    from concourse import library_config
    from concourse.masks import make_identity

    Bl, Bh = meta["Bl"], meta["Bh"]
    ntiles = sum(Bl) + sum(Bh)
    nslots = ntiles * 128
    maxW = max((Bl[b] + Bh[b]) * 128 for b in range(NB))
    dt = mybir.dt
    AF = mybir.ActivationFunctionType
    OP = mybir.AluOpType

    nc = bacc.Bacc("TRN2", target_bir_lowering=False, debug=debug,
                   num_devices=NCORES)

    ext_in = lambda n, s, d: nc.dram_tensor(n, s, d, kind="ExternalInput")
    x_sh_t = ext_in("x_sh_t", [IN_DIM, SHP], dt.bfloat16)
    idx_src_d = ext_in("idx_src", [16, nslots // 16], dt.int16)
    tgt_rel_d = ext_in("tgt_rel", [128, ntiles], dt.int8)
    cinv_d = ext_in("cinv_n", [1, SHP], dt.float32)
    wbf_d = ext_in("wbf_sh", [_WBF_SH], dt.bfloat16)
    bf32_d = ext_in("bf32", [_BF32_TOTAL], dt.float32)
    y_d = nc.dram_tensor("y", [1, SH], dt.float32, kind="ExternalOutput")

    wbf_stage = nc.dram_tensor("wbf_stage", [_WBF_SH], dt.bfloat16)
    idx_tgt_hbm = nc.dram_tensor("idx_tgt_hbm", [nslots], dt.int16)
    wbf_full = nc.dram_tensor("wbf_full", [_WBF_SH * NCORES], dt.bfloat16,
                              addr_space="Shared")
    table = nc.dram_tensor("table", [N, D], dt.bfloat16, addr_space="Shared")
    loc_tab = nc.dram_tensor("loc_tab", [SHP, D], dt.bfloat16)
    cc_in = nc.dram_tensor("cc_in", [SH, D], dt.bfloat16)

    groups512 = lambda W: [(g0, min(512, W - g0)) for g0 in range(0, W, 512)]
    as3d = lambda ap: ap.rearrange("p (o n) -> p o n", o=1)

    with tile.TileContext(nc, num_cores=NCORES) as tc:
        nc.gpsimd.load_library(library_config.mlp)

        with (
            tc.tile_pool(name="const", bufs=1) as cpool,
            tc.tile_pool(name="state", bufs=1) as spool,
            tc.tile_pool(name="gather", bufs=2) as gpool,
            tc.tile_pool(name="mlp", bufs=2) as mpool,
            tc.tile_pool(name="psA", bufs=1, space="PSUM") as ppA,
            tc.tile_pool(name="psB", bufs=1, space="PSUM") as ppB,
            tc.tile_pool(name="psC", bufs=2, space="PSUM") as ppC,
        ):
            # Distribute the bf16 weight blob: each core contributes 1/8.
            # (Collectives cannot read IO tensors; bounce through an
            # internal DRAM staging tensor.)
            nc.sync.dma_start(out=wbf_stage.ap(), in_=wbf_d.ap())
            nc.gpsimd.collective_compute(
                "AllGather", OP.bypass,
                replica_groups=[list(range(NCORES))],
                ins=[wbf_stage.ap()], outs=[wbf_full.ap()])

            def _blob_off(layout, nm):
                off = 0
                for n, (p, k) in layout:
                    if n == nm:
                        return off, p, k
                    off += p * k
                raise KeyError(nm)

            def wld(nm):
                off, p, k = _blob_off(_WBF_LAYOUT, nm)
                t = cpool.tile([p, k], dt.bfloat16, tag=nm)
                src = wbf_full.ap()[off:off + p * k]
                nc.sync.dma_start(out=t[:], in_=src.rearrange("(p k) -> p k", p=p))
                return t

            def bld(nm):
                off, p, k = _blob_off(_BF32_LAYOUT, nm)
                t = cpool.tile([p, k], dt.float32, tag=nm)
                src = bf32_d.ap()[off:off + p * k]
                nc.sync.dma_start(out=t[:], in_=src.rearrange("(p k) -> p k", p=p))
                return t

            def ld(dram_ap, nm):
                t = cpool.tile(list(dram_ap.shape), dram_ap.dtype, tag=nm)
                nc.sync.dma_start(out=t[:], in_=dram_ap)
                return t

            # Gather indices: stage compact [16, W], replicate into the
            # 128-partition wrap layout on device.
            W16 = nslots // 16
            idx_src = cpool.tile([128, W16], dt.int16, tag="idx_src")
            for g in range(8):
                nc.sync.dma_start(out=idx_src[16 * g:16 * (g + 1), :],
                                  in_=idx_src_d.ap())

            tgt_rel_i8 = ld(tgt_rel_d.ap(), "tgt_rel_i8")
            tgt_rel = cpool.tile([128, ntiles], dt.float32, tag="tgt_rel")
            nc.vector.tensor_copy(tgt_rel[:], tgt_rel_i8[:])

            # Derive the target-gather indices on device:
            # idx_tgt[slot] = max(tgt_rel[slot], 0) + 128*block(slot)
            # (clamp padding to a safe row; psel zeroes its contribution),
            # then shuffle from the [slot%128, slot//128] layout into the
            # dma_gather wrap layout [slot%16, slot//16] via a DRAM bounce,
            # replicated into the 8 partition groups.
            tgtf = cpool.tile([128, ntiles], dt.float32, tag="tgtf")
            tgt16 = cpool.tile([128, ntiles], dt.int16, tag="tgt16")
            t0 = 0
            for b in range(NB):
                t1 = t0 + Bl[b] + Bh[b]
                nc.vector.tensor_scalar(tgtf[:, t0:t1], tgt_rel[:, t0:t1],
                                        0.0, float(128 * b), OP.max, OP.add)
                t0 = t1
            assert t0 == ntiles
            nc.vector.tensor_copy(tgt16[:], tgtf[:])
            nc.sync.dma_start(
                out=idx_tgt_hbm.ap().rearrange("(t p) -> p t", p=128),
                in_=tgt16[:])
            idx_tgt = cpool.tile([128, W16], dt.int16, tag="idx_tgt")
            for g in range(8):
                nc.sync.dma_start(
                    out=idx_tgt[16 * g:16 * (g + 1), :],
                    in_=idx_tgt_hbm.ap().rearrange("(c r) -> r c", r=16))

            xsh = ld(x_sh_t.ap(), "xsh")

            wenc = wld("wenc")
            w1t, w1s, w2a, w2b, w3a, w3b, wih, whh = ([] for _ in range(8))
            for l in range(LAYERS):
                w1t.append(wld(f"w1t{l}"))
                w1s.append(wld(f"w1s{l}"))
                w2a.append(wld(f"w2a{l}"))
                w2b.append(wld(f"w2b{l}"))
                w3a.append(wld(f"w3a{l}"))
                w3b.append(wld(f"w3b{l}"))
                wih.append(wld(f"wih{l}"))
                whh.append(wld(f"whh{l}"))
            wd1 = wld("wd1")
            wd2a = wld("wd2a")
            wd2b = wld("wd2b")
            wd3a = wld("wd3a")
            wd3b = wld("wd3b")

            benc = bld("benc")
            b1, b2, brz, bgin, bghn = ([] for _ in range(5))
            for l in range(LAYERS):
                b1.append(bld(f"b1{l}"))
                b2.append(bld(f"b2{l}"))
                brz.append(bld(f"brz{l}"))
                bgin.append(bld(f"bgin{l}"))
                bghn.append(bld(f"bghn{l}"))
            bd1 = bld("bd1")
            bd2 = bld("bd2")

            iota = cpool.tile([128, 128], dt.float32, tag="iota")
            nc.gpsimd.iota(iota[:], pattern=[[1, 128]], base=0,
                           channel_multiplier=0,
                           allow_small_or_imprecise_dtypes=True)
            ident = cpool.tile([128, 128], dt.bfloat16, tag="ident")
            make_identity(nc, ident[:])
            ones1 = cpool.tile([1, 128], dt.float32, tag="ones1")
            nc.vector.memset(ones1[:], 1.0)

            # Per-node 1/count, broadcast across all partitions once:
            # cinv_bc[:, n] = 1/count[n] via chunked rank-1 PE products.
            cinv_bc = cpool.tile([128, SHP], dt.bfloat16, tag="cinv_bc")
            for n0, w in groups512(SHP):
                ct = cpool.tile([1, 512], dt.float32, tag="cchunk")
                nc.sync.dma_start(out=ct[:, :w], in_=cinv_d.ap()[:, n0:n0 + w])
                p = ppA.tile([128, 1024], dt.float32, tag="mp0")
                nc.tensor.matmul(p[:, :w], lhsT=ones1[:], rhs=ct[:, :w],
                                 start=True, stop=True)
                nc.vector.tensor_copy(cinv_bc[:, n0:n0 + w], p[:, :w])

            h_f32 = spool.tile([128, SHP], dt.float32, tag="h_f32")
            h_bf = spool.tile([128, SHP], dt.bfloat16, tag="h_bf")
            h_nm = spool.tile([128, SHP], dt.bfloat16, tag="h_nm")
            agg_bf = spool.tile([128, SHP], dt.bfloat16, tag="agg_bf")

            def finish_layer():
                for b in range(NB):
                    tp = ppB.tile([128, 128], dt.bfloat16, tag="m3p")
                    nc.tensor.transpose(tp[:], h_bf[:, b * 128:(b + 1) * 128],
                                        ident[:])
                    nc.vector.tensor_copy(h_nm[:, b * 128:(b + 1) * 128], tp[:])
                nc.sync.dma_start(
                    out=loc_tab.ap().rearrange("(b p) d -> p b d", p=128),
                    in_=h_nm[:].rearrange("p (b d) -> p b d", d=D))
                nbf = SH // 128  # full 128-node blocks in the shard
                nc.sync.dma_start(
                    out=cc_in.ap()[:nbf * 128].rearrange("(b p) d -> p b d", p=128),
                    in_=h_nm[:, :nbf * 128].rearrange("p (b d) -> p b d", d=D))
                if SH > nbf * 128:
                    nc.sync.dma_start(
                        out=cc_in.ap()[nbf * 128:SH],
                        in_=h_nm[:SH - nbf * 128, nbf * 128:(nbf + 1) * 128])
                nc.gpsimd.collective_compute(
                    "AllGather", OP.bypass,
                    replica_groups=[list(range(NCORES))],
                    ins=[cc_in.ap()], outs=[table.ap()])

            # ---------------- encoder ----------------
            for n0, w in groups512(SHP):
                ps = ppA.tile([128, 512], dt.float32, tag="mp0")
                nc.tensor.matmul(ps[:, :w], lhsT=wenc[:], rhs=xsh[:, n0:n0 + w],
                                 start=True, stop=True)
                nc.scalar.activation(h_f32[:, n0:n0 + w], ps[:, :w], AF.Tanh,
                                     bias=benc[:, 0:1])
                nc.vector.tensor_copy(h_bf[:, n0:n0 + w], h_f32[:, n0:n0 + w])
            finish_layer()

            # ---------------- message-passing layers ----------------
            # repeat>1 re-runs the layer stack for timing (garbage numerics
            # after the first pass; used only by the benchmark).
            for l in [l for _ in range(repeat) for l in range(LAYERS)]:
                tile_idx = 0
                slot_off = 0
                for b in range(NB):
                    wl, wh = Bl[b] * 128, Bh[b] * 128
                    W = wl + wh
                    gsrc = gpool.tile([128, maxW], dt.bfloat16, tag="gsrc")
                    gtgt = gpool.tile([128, maxW], dt.bfloat16, tag="gtgt")
                    nc.gpsimd.dma_gather(
                        as3d(gsrc[:, 0:wl]), table.ap()[0:SPLIT],
                        idx_src[:, slot_off:slot_off + wl // 16],
                        wl, wl, D, transpose=True, single_packet=False)
                    nc.gpsimd.dma_gather(
                        as3d(gsrc[:, wl:W]), table.ap()[SPLIT:N],
                        idx_src[:, slot_off + wl // 16:slot_off + W // 16],
                        wh, wh, D, transpose=True, single_packet=False)
                    nc.gpsimd.dma_gather(
                        as3d(gtgt[:, 0:W]), loc_tab.ap(),
                        idx_tgt[:, slot_off:slot_off + W // 16],
                        W, W, D, transpose=True, single_packet=False)
                    slot_off += W // 16

                    aggp = ppC.tile([128, 128], dt.float32, tag="aggp")
                    first_tile = 0
                    for g0 in range(0, W, 1024):
                        w = min(1024, W - g0)
                        nt = w // 128
                        halves = [(h0, min(512, w - h0))
                                  for h0 in range(0, w, 512)]
                        m1s, m2s = [], []
                        for ci in range(2):
                            cs = slice(ci * 128, (ci + 1) * 128)
                            p = ppA.tile([128, 1024], dt.float32, tag=f"mp{ci}")
                            for h0, hw in halves:
                                nc.tensor.matmul(
                                    p[:, h0:h0 + hw], lhsT=w1t[l][:, cs],
                                    rhs=gtgt[:, g0 + h0:g0 + h0 + hw],
                                    start=True, stop=False)
                            for h0, hw in halves:
                                nc.tensor.matmul(
                                    p[:, h0:h0 + hw], lhsT=w1s[l][:, cs],
                                    rhs=gsrc[:, g0 + h0:g0 + h0 + hw],
                                    start=False, stop=True)
                            s = mpool.tile([128, 1024], dt.bfloat16,
                                           tag=f"m1s{ci}")
                            nc.scalar.activation(s[:, :w], p[:, :w], AF.Tanh,
                                                 bias=b1[l][:, ci:ci + 1])
                            m1s.append(s)
                        for ci in range(2):
                            cs = slice(ci * 128, (ci + 1) * 128)
                            p = ppA.tile([128, 1024], dt.float32, tag=f"mp{ci}")
                            for h0, hw in halves:
                                nc.tensor.matmul(
                                    p[:, h0:h0 + hw], lhsT=w2a[l][:, cs],
                                    rhs=m1s[0][:, h0:h0 + hw],
                                    start=True, stop=False)
                            for h0, hw in halves:
                                nc.tensor.matmul(
                                    p[:, h0:h0 + hw], lhsT=w2b[l][:, cs],
                                    rhs=m1s[1][:, h0:h0 + hw],
                                    start=False, stop=True)
                            s = mpool.tile([128, 1024], dt.bfloat16,
                                           tag=f"m2s{ci}")
                            nc.scalar.activation(s[:, :w], p[:, :w], AF.Tanh,
                                                 bias=b2[l][:, ci:ci + 1])
                            m2s.append(s)
                        m3p = ppB.tile([128, 1024], dt.float32, tag="m3p")
                        for t in range(nt):
                            ts = slice(t * 128, (t + 1) * 128)
                            nc.tensor.matmul(m3p[:, ts], lhsT=m2s[0][:, ts],
                                             rhs=w3a[l][:], start=True, stop=False)
                            nc.tensor.matmul(m3p[:, ts], lhsT=m2s[1][:, ts],
                                             rhs=w3b[l][:], start=False, stop=True)
                        m3s = mpool.tile([128, 1024], dt.bfloat16, tag="m3s")
                        nc.vector.tensor_copy(m3s[:, :w], m3p[:, :w])
                        psel = mpool.tile([128, 1024], dt.bfloat16, tag="psel")
                        for t in range(nt):
                            col = tile_idx + first_tile + t
                            nc.vector.tensor_scalar(
                                psel[:, t * 128:(t + 1) * 128], iota[:],
                                tgt_rel[:, col:col + 1], None, OP.is_equal)
                        for t in range(nt):
                            ts = slice(t * 128, (t + 1) * 128)
                            nc.tensor.matmul(
                                aggp[:], lhsT=m3s[:, ts], rhs=psel[:, ts],
                                start=(first_tile + t == 0),
                                stop=(first_tile + t == W // 128 - 1))
                        first_tile += nt
                    tile_idx += W // 128
                    # agg holds unscaled segment sums; scale columns by
                    # the pre-broadcast 1/count table.
                    nc.vector.tensor_tensor(
                        agg_bf[:, b * 128:(b + 1) * 128], aggp[:],
                        cinv_bc[:, b * 128:(b + 1) * 128], OP.mult)

                # ---- GRU update over the node shard ----
                for n0, w in groups512(SHP):
                    ns = slice(n0, n0 + w)
                    rz = []
                    for k in range(2):
                        ks = slice(k * 128, (k + 1) * 128)
                        p = ppA.tile([128, 512], dt.float32, tag=f"mp{k}")
                        nc.tensor.matmul(p[:, :w], lhsT=wih[l][:, ks],
                                         rhs=agg_bf[:, ns], start=True, stop=False)
                        nc.tensor.matmul(p[:, :w], lhsT=whh[l][:, ks],
                                         rhs=h_bf[:, ns], start=False, stop=True)
                        s = mpool.tile([128, 512], dt.bfloat16, tag=f"m1s{k}")
                        nc.scalar.activation(s[:, :w], p[:, :w], AF.Sigmoid,
                                             bias=brz[l][:, k:k + 1])
                        rz.append(s)
                    gin = ppB.tile([128, 512], dt.float32, tag="m3p")
                    nc.tensor.matmul(gin[:, :w], lhsT=wih[l][:, 2 * 128:],
                                     rhs=agg_bf[:, ns], start=True, stop=True)
                    ghn = ppA.tile([128, 512], dt.float32, tag="mp0")
                    nc.tensor.matmul(ghn[:, :w], lhsT=whh[l][:, 2 * 128:],
                                     rhs=h_bf[:, ns], start=True, stop=True)
                    ghnb = mpool.tile([128, 512], dt.float32, tag="m2s0")
                    nc.vector.tensor_scalar(ghnb[:, :w], ghn[:, :w],
                                            bghn[l][:, 0:1], None, OP.add)
                    t1 = mpool.tile([128, 512], dt.float32, tag="m2s1")
                    nc.vector.tensor_tensor(t1[:, :w], rz[0][:, :w], ghnb[:, :w],
                                            OP.mult)
                    pre = mpool.tile([128, 512], dt.float32, tag="m3s")
                    nc.vector.tensor_tensor(pre[:, :w], gin[:, :w], t1[:, :w],
                                            OP.add)
                    nn = mpool.tile([128, 512], dt.float32, tag="psel")
                    nc.scalar.activation(nn[:, :w], pre[:, :w], AF.Tanh,
                                         bias=bgin[l][:, 0:1])
                    dd = mpool.tile([128, 512], dt.float32, tag="dd")
                    nc.vector.tensor_tensor(dd[:, :w], h_f32[:, ns], nn[:, :w],
                                            OP.subtract)
                    ee = mpool.tile([128, 512], dt.float32, tag="ee")
                    nc.vector.tensor_tensor(ee[:, :w], rz[1][:, :w], dd[:, :w],
                                            OP.mult)
                    nc.vector.tensor_tensor(h_f32[:, ns], nn[:, :w], ee[:, :w],
                                            OP.add)
                    nc.vector.tensor_copy(h_bf[:, ns], h_f32[:, ns])

                if l < LAYERS - 1:
                    finish_layer()

            # ---------------- decoder ----------------
            for n0, w in groups512(SHP):
                ns = slice(n0, n0 + w)
                o1s, o2s = [], []
                for ci in range(2):
                    cs = slice(ci * 128, (ci + 1) * 128)
                    p = ppA.tile([128, 512], dt.float32, tag=f"mp{ci}")
                    nc.tensor.matmul(p[:, :w], lhsT=wd1[:, cs],
                                     rhs=h_bf[:, ns], start=True, stop=True)
                    s = mpool.tile([128, 512], dt.bfloat16, tag=f"m1s{ci}")
                    nc.scalar.activation(s[:, :w], p[:, :w], AF.Tanh,
                                         bias=bd1[:, ci:ci + 1])
                    o1s.append(s)
                for ci in range(2):
                    cs = slice(ci * 128, (ci + 1) * 128)
                    p = ppA.tile([128, 512], dt.float32, tag=f"mp{ci}")
                    nc.tensor.matmul(p[:, :w], lhsT=wd2a[:, cs],
                                     rhs=o1s[0][:, :w], start=True, stop=False)
                    nc.tensor.matmul(p[:, :w], lhsT=wd2b[:, cs],
                                     rhs=o1s[1][:, :w], start=False, stop=True)
                    s = mpool.tile([128, 512], dt.bfloat16, tag=f"m2s{ci}")
                    nc.scalar.activation(s[:, :w], p[:, :w], AF.Tanh,
                                         bias=bd2[:, ci:ci + 1])
                    o2s.append(s)
                o3p = ppB.tile([1, 512], dt.float32, tag="m3p")
                nc.tensor.matmul(o3p[:, :w], lhsT=wd3a[:], rhs=o2s[0][:, :w],
                                 start=True, stop=False)
                nc.tensor.matmul(o3p[:, :w], lhsT=wd3b[:], rhs=o2s[1][:, :w],
                                 start=False, stop=True)
                yt = mpool.tile([1, 512], dt.float32, tag="m3s")
                nc.scalar.copy(yt[:, :w], o3p[:, :w])
                we = min(w, SH - n0) if n0 < SH else 0
                if we > 0:
                    nc.sync.dma_start(out=y_d.ap()[:, n0:n0 + we],
                                      in_=yt[:, :we])

    nc.compile()
    return nc


# ----------------------------------------------------------------------------
# Entry point
# ----------------------------------------------------------------------------

def make_in_maps(per_core, w):
    in_maps = []
    for c in range(NCORES):
        m = dict(per_core[c])
        m["wbf_sh"] = np.ascontiguousarray(w["wbf"][c * _WBF_SH:(c + 1) * _WBF_SH])
        m["bf32"] = w["bf32"]
        in_maps.append(m)
    return in_maps


def kernel(**inputs) -> np.ndarray:
    from concourse.bass_utils import run_bass_kernel_spmd

    meta, per_core = _prep(np.asarray(inputs["x"], np.float32),
                           np.asarray(inputs["edge_index"]))
    w = _prep_weights(inputs)

    key = (tuple(meta["Bl"]), tuple(meta["Bh"]))
    if key not in _PROGRAM_CACHE:
        _PROGRAM_CACHE[key] = _build_program(meta)
    nc = _PROGRAM_CACHE[key]

    in_maps = make_in_maps(per_core, w)
    res = run_bass_kernel_spmd(nc, in_maps, core_ids=list(range(NCORES)),
                               **_RUN_KWARGS)
    global _LAST_RESULTS
    _LAST_RESULTS = res
    out = np.concatenate([res.results[c]["y"][0] for c in range(NCORES)])
    return (out + np.asarray(inputs["dec_b3"], np.float32)[0]).astype(np.float32)


# revision 38
# speedup vs baseline: 1.0412x; 1.0412x over previous
"""Trainium2 Bass kernel for nn_MessagePassingGNN (8-core SPMD).

Strategy:
  - Sort edges (with self-loops) by target node; shard TARGET NODES across
    the 8 cores (6250 each) so each core owns a contiguous edge range and
    the segment-sum aggregation is core-local (no all-reduce).
  - Per layer, each core gathers source-node features from a replicated
    bf16 feature table in DRAM via dma_gather(transpose=True), which yields
    feature-major tiles that feed the message-MLP matmuls directly (no
    on-chip transposes). Target-side gathers read a core-local shard table
    so they never wait on the collective.
  - The scatter-mean aggregation runs on the tensor engine: a scaled one-hot
    matrix P[e, n] = (tgt_rel[e] == n) / count[tgt_e] is built by one fused
    DVE tensor_scalar per 128-edge tile, then agg += m3_tile.T @ P_tile
    accumulates in PSUM per 128-target-node block.
  - GRU update is node-sharded; updated shard features are AllGather'd into
    every core's table for the next layer. The decoder runs on the local
    shard; the host concatenates the 8 shards.

Host->device staging is minimized (~0.82 MB/core vs ~6.8 MB for the naive
layout): source-gather indices are staged compactly ([16, W] int16,
replicated into the 128-partition wrap layout on device), target-gather
indices are derived on device from the int8 tgt_rel table (clamped pad
slots + 128*block, shuffled into wrap layout via a DRAM bounce), the
scatter-mean scale is a per-node [1, SHP] f32 row broadcast across
partitions once via chunked rank-1 PE products, and all bf16 weights are
packed into one blob that is sharded 1/8th per core and AllGather'd on
device. The unscaled one-hot P[e, n] = (tgt_rel[e] == n) aggregates raw
segment sums; the 1/count scaling applies on the [d, n] PSUM result.

All matmuls are bf16 with fp32 PSUM accumulation; GRU elementwise math is
fp32. Host-measured end-to-end L2 relative error vs fp32 reference ~1e-2.
"""

import math

import numpy as np
import ml_dtypes

# Problem constants (hardcoded per harness contract).
N, IN_DIM, D, H, E, LAYERS = 50000, 16, 128, 256, 800000, 3
NCORES = 8
SH = N // NCORES            # 6250 nodes per shard
NB = (SH + 127) // 128      # 49 blocks of 128 target nodes
SHP = NB * 128              # 6272 padded shard width
SPLIT = 32768               # int16 index split for the gather table
BF16 = ml_dtypes.bfloat16

_PROGRAM_CACHE = {}
_RUN_KWARGS = {}       # test harness may set {"trace": True}
_LAST_RESULTS = None   # BassKernelResults of the most recent run

# Packed bf16 weight blob layout: (name, [P, K]) in order. Per-layer
# weights appear once per layer (l-major). Host packing and device
# unpacking both walk this table.
_WBF_LAYOUT = (
    [("wenc", [IN_DIM, 128])]
    + [
        (f"{nm}{l}", shp)
        for l in range(LAYERS)
        for nm, shp in [
            ("w1t", [128, H]), ("w1s", [128, H]),
            ("w2a", [128, H]), ("w2b", [128, H]),
            ("w3a", [128, D]), ("w3b", [128, D]),
            ("wih", [128, 3 * D]), ("whh", [128, 3 * D]),
        ]
    ]
    + [("wd1", [128, H]), ("wd2a", [128, H]), ("wd2b", [128, H]),
       ("wd3a", [128, 1]), ("wd3b", [128, 1])]
)
# f32 bias blob layout.
_BF32_LAYOUT = (
    [("benc", [128, 1])]
    + [
        (f"{nm}{l}", shp)
        for l in range(LAYERS)
        for nm, shp in [
            ("b1", [128, 2]), ("b2", [128, 2]), ("brz", [128, 2]),
            ("bgin", [128, 1]), ("bghn", [128, 1]),
        ]
    ]
    + [("bd1", [128, 2]), ("bd2", [128, 2])]
)
_WBF_TOTAL = sum(p * k for _, (p, k) in _WBF_LAYOUT)
_WBF_SH = -(-_WBF_TOTAL // NCORES)           # per-core slice (padded)
_BF32_TOTAL = sum(p * k for _, (p, k) in _BF32_LAYOUT)


# ----------------------------------------------------------------------------
# Host-side preprocessing
# ----------------------------------------------------------------------------

def _wrap_idx(idx_i16):
    """Compact dma_gather index layout: index i at [i % 16, i // 16].
    The 8x partition-group replication happens on device."""
    n = idx_i16.shape[0]
    return np.ascontiguousarray(idx_i16.reshape(n // 16, 16).T)


def _prep(x, edge_index):
    loops = np.arange(N, dtype=np.int64)
    src = np.concatenate([np.asarray(edge_index[0]), loops])
    tgt = np.concatenate([np.asarray(edge_index[1]), loops])
    order = np.argsort(tgt, kind="stable")
    src_s = src[order].astype(np.int32)
    tgt_s = tgt[order].astype(np.int32)
    counts = np.zeros(N, np.float32)
    np.add.at(counts, tgt_s, 1.0)
    cinv_n = (1.0 / counts).astype(np.float32)

    node_starts = np.searchsorted(tgt_s, np.arange(N + 1))
    lows = np.zeros((NCORES, NB), np.int64)
    highs = np.zeros((NCORES, NB), np.int64)
    rng = {}
    for c in range(NCORES):
        for b in range(NB):
            lo_node = c * SH + b * 128
            hi_node = min(c * SH + SH, lo_node + 128)
            e0, e1 = node_starts[lo_node], node_starts[hi_node]
            nl = int((src_s[e0:e1] < SPLIT).sum())
            lows[c, b] = nl
            highs[c, b] = (e1 - e0) - nl
            rng[(c, b)] = (e0, e1)
    Bl = [int(max(1, math.ceil(lows[:, b].max() / 128))) for b in range(NB)]
    Bh = [int(max(1, math.ceil(highs[:, b].max() / 128))) for b in range(NB)]

    meta = {"Bl": Bl, "Bh": Bh}
    ntiles = sum(Bl) + sum(Bh)
    nslots = ntiles * 128

    per_core = []
    for c in range(NCORES):
        idx_src = np.zeros(nslots, np.int16)
        tgt_rel = np.full(nslots, -1, np.int8)
        off = 0
        for b in range(NB):
            e0, e1 = rng[(c, b)]
            s, t = src_s[e0:e1], tgt_s[e0:e1]
            lo = s < SPLIT
            for mask, cap, base in ((lo, Bl[b], 0), (~lo, Bh[b], SPLIT)):
                sh_, th_ = s[mask], t[mask]
                n = sh_.shape[0]
                idx_src[off:off + n] = (sh_ - base).astype(np.int16)
                tgt_rel[off:off + n] = (th_ - (c * SH + b * 128)).astype(np.int8)
                off += cap * 128
        assert off == nslots

        src_cols = []
        off = 0
        for b in range(NB):
            wl, wh = Bl[b] * 128, Bh[b] * 128
            src_cols.append(_wrap_idx(idx_src[off:off + wl]))
            src_cols.append(_wrap_idx(idx_src[off + wl:off + wl + wh]))
            off += wl + wh

        xs = np.zeros((IN_DIM, SHP), np.float32)
        xs[:, :SH] = np.asarray(x[c * SH:(c + 1) * SH]).T
        cn = np.zeros((1, SHP), np.float32)
        cn[0, :SH] = cinv_n[c * SH:(c + 1) * SH]
        per_core.append({
            "x_sh_t": xs.astype(BF16),
            "idx_src": np.concatenate(src_cols, axis=1),
            "tgt_rel": tgt_rel.reshape(ntiles, 128).T.copy(),
            "cinv_n": cn,
        })
    return meta, per_core


def _prep_weights(inp):
    f32 = np.float32
    bf = lambda a: np.ascontiguousarray(np.asarray(a, f32)).astype(BF16)
    w = {}
    w["wenc"] = bf(inp["enc_W"])
    w["benc"] = np.asarray(inp["enc_b"], f32).reshape(128, 1)
    for l in range(LAYERS):
        w[f"w1t{l}"] = bf(inp["msg_W1"][l, :D, :])
        w[f"w1s{l}"] = bf(inp["msg_W1"][l, D:, :])
        w[f"w2a{l}"] = bf(inp["msg_W2"][l, :128, :])
        w[f"w2b{l}"] = bf(inp["msg_W2"][l, 128:, :])
        w[f"w3a{l}"] = bf(inp["msg_W3"][l, :128, :])
        w[f"w3b{l}"] = bf(inp["msg_W3"][l, 128:, :])
        w[f"wih{l}"] = bf(inp["gru_Wih"][l])
        w[f"whh{l}"] = bf(inp["gru_Whh"][l])
        w[f"b1{l}"] = np.asarray(inp["msg_b1"][l], f32).reshape(2, 128).T
        w[f"b2{l}"] = np.asarray(inp["msg_b2"][l], f32).reshape(2, 128).T
        bgi = (np.asarray(inp["msg_b3"][l], f32)
               @ np.asarray(inp["gru_Wih"][l], f32)
               + np.asarray(inp["gru_bih"][l], f32))
        bhh = np.asarray(inp["gru_bhh"][l], f32)
        w[f"brz{l}"] = (bgi[:2 * D] + bhh[:2 * D]).reshape(2, 128).T
        w[f"bgin{l}"] = bgi[2 * D:].reshape(128, 1)
        w[f"bghn{l}"] = bhh[2 * D:].reshape(128, 1)
    w["wd1"] = bf(inp["dec_W1"])
    w["wd2a"] = bf(inp["dec_W2"][:128, :])
    w["wd2b"] = bf(inp["dec_W2"][128:, :])
    w["wd3a"] = bf(inp["dec_W3"][:128, :])
    w["wd3b"] = bf(inp["dec_W3"][128:, :])
    w["bd1"] = np.asarray(inp["dec_b1"], f32).reshape(2, 128).T
    w["bd2"] = np.asarray(inp["dec_b2"], f32).reshape(2, 128).T

    wbf = np.zeros(_WBF_SH * NCORES, BF16)
    off = 0
    for nm, (p, k) in _WBF_LAYOUT:
        a = np.ascontiguousarray(w[nm])
        assert a.shape == (p, k) and a.dtype == BF16, (nm, a.shape, a.dtype)
        wbf[off:off + p * k] = a.ravel()
        off += p * k
    assert off == _WBF_TOTAL
    bf32 = np.zeros(_BF32_TOTAL, f32)
    off = 0
    for nm, (p, k) in _BF32_LAYOUT:
        a = np.ascontiguousarray(np.asarray(w[nm], f32))
        assert a.shape == (p, k), (nm, a.shape)
        bf32[off:off + p * k] = a.ravel()
        off += p * k
    assert off == _BF32_TOTAL
    return {"wbf": wbf, "bf32": bf32}


# ----------------------------------------------------------------------------
# Bass program
# ----------------------------------------------------------------------------

def _build_program(meta, debug=False, repeat=1):
    import concourse.bacc as bacc
    import concourse.mybir as mybir
    import concourse.tile as tile
    from concourse import library_config
    from concourse.masks import make_identity

    Bl, Bh = meta["Bl"], meta["Bh"]
    ntiles = sum(Bl) + sum(Bh)
    nslots = ntiles * 128
    maxW = max((Bl[b] + Bh[b]) * 128 for b in range(NB))
    dt = mybir.dt
    AF = mybir.ActivationFunctionType
    OP = mybir.AluOpType

    nc = bacc.Bacc("TRN2", target_bir_lowering=False, debug=debug,
                   num_devices=NCORES)

    ext_in = lambda n, s, d: nc.dram_tensor(n, s, d, kind="ExternalInput")
    x_sh_t = ext_in("x_sh_t", [IN_DIM, SHP], dt.bfloat16)
    idx_src_d = ext_in("idx_src", [16, nslots // 16], dt.int16)
    tgt_rel_d = ext_in("tgt_rel", [128, ntiles], dt.int8)
    cinv_d = ext_in("cinv_n", [1, SHP], dt.float32)
    wbf_d = ext_in("wbf_sh", [_WBF_SH], dt.bfloat16)
    bf32_d = ext_in("bf32", [_BF32_TOTAL], dt.float32)
    y_d = nc.dram_tensor("y", [1, SH], dt.float32, kind="ExternalOutput")

    wbf_stage = nc.dram_tensor("wbf_stage", [_WBF_SH], dt.bfloat16)
    idx_tgt_hbm = nc.dram_tensor("idx_tgt_hbm", [nslots], dt.int16)
    wbf_full = nc.dram_tensor("wbf_full", [_WBF_SH * NCORES], dt.bfloat16,
                              addr_space="Shared")
    table = nc.dram_tensor("table", [N, D], dt.bfloat16, addr_space="Shared")
    loc_tab = nc.dram_tensor("loc_tab", [SHP, D], dt.bfloat16)
    cc_in = nc.dram_tensor("cc_in", [SH, D], dt.bfloat16)

    groups512 = lambda W: [(g0, min(512, W - g0)) for g0 in range(0, W, 512)]
    as3d = lambda ap: ap.rearrange("p (o n) -> p o n", o=1)

    with tile.TileContext(nc, num_cores=NCORES) as tc:
        nc.gpsimd.load_library(library_config.mlp)

        with (
            tc.tile_pool(name="const", bufs=1) as cpool,
            tc.tile_pool(name="state", bufs=1) as spool,
            tc.tile_pool(name="gather", bufs=2) as gpool,
            tc.tile_pool(name="mlp", bufs=2) as mpool,
            tc.tile_pool(name="psA", bufs=1, space="PSUM") as ppA,
            tc.tile_pool(name="psB", bufs=1, space="PSUM") as ppB,
            tc.tile_pool(name="psC", bufs=2, space="PSUM") as ppC,
        ):
            # Distribute the bf16 weight blob: each core contributes 1/8.
            # (Collectives cannot read IO tensors; bounce through an
            # internal DRAM staging tensor.)
            nc.sync.dma_start(out=wbf_stage.ap(), in_=wbf_d.ap())
            nc.gpsimd.collective_compute(
                "AllGather", OP.bypass,
                replica_groups=[list(range(NCORES))],
                ins=[wbf_stage.ap()], outs=[wbf_full.ap()])

            def _blob_off(layout, nm):
                off = 0
                for n, (p, k) in layout:
                    if n == nm:
                        return off, p, k
                    off += p * k
                raise KeyError(nm)

            def wld(nm):
                off, p, k = _blob_off(_WBF_LAYOUT, nm)
                t = cpool.tile([p, k], dt.bfloat16, tag=nm)
                src = wbf_full.ap()[off:off + p * k]
                nc.sync.dma_start(out=t[:], in_=src.rearrange("(p k) -> p k", p=p))
                return t

            def bld(nm):
                off, p, k = _blob_off(_BF32_LAYOUT, nm)
                t = cpool.tile([p, k], dt.float32, tag=nm)
                src = bf32_d.ap()[off:off + p * k]
                nc.sync.dma_start(out=t[:], in_=src.rearrange("(p k) -> p k", p=p))
                return t

            def ld(dram_ap, nm):
                t = cpool.tile(list(dram_ap.shape), dram_ap.dtype, tag=nm)
                nc.sync.dma_start(out=t[:], in_=dram_ap)
                return t

            # Gather indices: stage compact [16, W], replicate into the
            # 128-partition wrap layout on device.
            W16 = nslots // 16
            idx_src = cpool.tile([128, W16], dt.int16, tag="idx_src")
            for g in range(8):
                nc.sync.dma_start(out=idx_src[16 * g:16 * (g + 1), :],
                                  in_=idx_src_d.ap())

            tgt_rel_i8 = ld(tgt_rel_d.ap(), "tgt_rel_i8")
            tgt_rel = cpool.tile([128, ntiles], dt.float32, tag="tgt_rel")
            nc.vector.tensor_copy(tgt_rel[:], tgt_rel_i8[:])

            # Derive the target-gather indices on device:
            # idx_tgt[slot] = max(tgt_rel[slot], 0) + 128*block(slot)
            # (clamp padding to a safe row; psel zeroes its contribution),
            # then shuffle from the [slot%128, slot//128] layout into the
            # dma_gather wrap layout [slot%16, slot//16] via a DRAM bounce,
            # replicated into the 8 partition groups.
            tgtf = cpool.tile([128, ntiles], dt.float32, tag="tgtf")
            tgt16 = cpool.tile([128, ntiles], dt.int16, tag="tgt16")
            t0 = 0
            for b in range(NB):
                t1 = t0 + Bl[b] + Bh[b]
                nc.vector.tensor_scalar(tgtf[:, t0:t1], tgt_rel[:, t0:t1],
                                        0.0, float(128 * b), OP.max, OP.add)
                t0 = t1
            assert t0 == ntiles
            nc.vector.tensor_copy(tgt16[:], tgtf[:])
            nc.sync.dma_start(
                out=idx_tgt_hbm.ap().rearrange("(t p) -> p t", p=128),
                in_=tgt16[:])
            idx_tgt = cpool.tile([128, W16], dt.int16, tag="idx_tgt")
            for g in range(8):
                nc.sync.dma_start(
                    out=idx_tgt[16 * g:16 * (g + 1), :],
                    in_=idx_tgt_hbm.ap().rearrange("(c r) -> r c", r=16))

            xsh = ld(x_sh_t.ap(), "xsh")

            wenc = wld("wenc")
            w1t, w1s, w2a, w2b, w3a, w3b, wih, whh = ([] for _ in range(8))
            for l in range(LAYERS):
                w1t.append(wld(f"w1t{l}"))
                w1s.append(wld(f"w1s{l}"))
                w2a.append(wld(f"w2a{l}"))
                w2b.append(wld(f"w2b{l}"))
                w3a.append(wld(f"w3a{l}"))
                w3b.append(wld(f"w3b{l}"))
                wih.append(wld(f"wih{l}"))
                whh.append(wld(f"whh{l}"))
            wd1 = wld("wd1")
            wd2a = wld("wd2a")
            wd2b = wld("wd2b")
            wd3a = wld("wd3a")
            wd3b = wld("wd3b")

            benc = bld("benc")
            b1, b2, brz, bgin, bghn = ([] for _ in range(5))
            for l in range(LAYERS):
                b1.append(bld(f"b1{l}"))
                b2.append(bld(f"b2{l}"))
                brz.append(bld(f"brz{l}"))
                bgin.append(bld(f"bgin{l}"))
                bghn.append(bld(f"bghn{l}"))
            bd1 = bld("bd1")
            bd2 = bld("bd2")

            iota = cpool.tile([128, 128], dt.float32, tag="iota")
            nc.gpsimd.iota(iota[:], pattern=[[1, 128]], base=0,
                           channel_multiplier=0,
                           allow_small_or_imprecise_dtypes=True)
            ident = cpool.tile([128, 128], dt.bfloat16, tag="ident")
            make_identity(nc, ident[:])
            ones1 = cpool.tile([1, 128], dt.float32, tag="ones1")
            nc.vector.memset(ones1[:], 1.0)

            # Per-node 1/count, broadcast across all partitions once:
            # cinv_bc[:, n] = 1/count[n] via chunked rank-1 PE products.
            cinv_bc = cpool.tile([128, SHP], dt.bfloat16, tag="cinv_bc")
            for n0, w in groups512(SHP):
                ct = cpool.tile([1, 512], dt.float32, tag="cchunk")
                nc.sync.dma_start(out=ct[:, :w], in_=cinv_d.ap()[:, n0:n0 + w])
                p = ppA.tile([128, 1024], dt.float32, tag="mp0")
                nc.tensor.matmul(p[:, :w], lhsT=ones1[:], rhs=ct[:, :w],
                                 start=True, stop=True)
                nc.vector.tensor_copy(cinv_bc[:, n0:n0 + w], p[:, :w])

            h_f32 = spool.tile([128, SHP], dt.float32, tag="h_f32")
            h_bf = spool.tile([128, SHP], dt.bfloat16, tag="h_bf")
            h_nm = spool.tile([128, SHP], dt.bfloat16, tag="h_nm")
            agg_bf = spool.tile([128, SHP], dt.bfloat16, tag="agg_bf")

            def finish_layer():
                for b in range(NB):
                    tp = ppB.tile([128, 128], dt.bfloat16, tag="m3p")
                    nc.tensor.transpose(tp[:], h_bf[:, b * 128:(b + 1) * 128],
                                        ident[:])
                    nc.vector.tensor_copy(h_nm[:, b * 128:(b + 1) * 128], tp[:])
                nc.sync.dma_start(
                    out=loc_tab.ap().rearrange("(b p) d -> p b d", p=128),
                    in_=h_nm[:].rearrange("p (b d) -> p b d", d=D))
                nbf = SH // 128  # full 128-node blocks in the shard
                nc.sync.dma_start(
                    out=cc_in.ap()[:nbf * 128].rearrange("(b p) d -> p b d", p=128),
                    in_=h_nm[:, :nbf * 128].rearrange("p (b d) -> p b d", d=D))
                if SH > nbf * 128:
                    nc.sync.dma_start(
                        out=cc_in.ap()[nbf * 128:SH],
                        in_=h_nm[:SH - nbf * 128, nbf * 128:(nbf + 1) * 128])
                nc.gpsimd.collective_compute(
                    "AllGather", OP.bypass,
                    replica_groups=[list(range(NCORES))],
                    ins=[cc_in.ap()], outs=[table.ap()])

            # ---------------- encoder ----------------
            for n0, w in groups512(SHP):
                ps = ppA.tile([128, 512], dt.float32, tag="mp0")
                nc.tensor.matmul(ps[:, :w], lhsT=wenc[:], rhs=xsh[:, n0:n0 + w],
                                 start=True, stop=True)
                nc.scalar.activation(h_f32[:, n0:n0 + w], ps[:, :w], AF.Tanh,
                                     bias=benc[:, 0:1])
                nc.vector.tensor_copy(h_bf[:, n0:n0 + w], h_f32[:, n0:n0 + w])
            finish_layer()

            # ---------------- message-passing layers ----------------
            # repeat>1 re-runs the layer stack for timing (garbage numerics
            # after the first pass; used only by the benchmark).
            for l in [l for _ in range(repeat) for l in range(LAYERS)]:
                tile_idx = 0
                slot_off = 0
                for b in range(NB):
                    wl, wh = Bl[b] * 128, Bh[b] * 128
                    W = wl + wh
                    gsrc = gpool.tile([128, maxW], dt.bfloat16, tag="gsrc")
                    gtgt = gpool.tile([128, maxW], dt.bfloat16, tag="gtgt")
                    nc.gpsimd.dma_gather(
                        as3d(gsrc[:, 0:wl]), table.ap()[0:SPLIT],
                        idx_src[:, slot_off:slot_off + wl // 16],
                        wl, wl, D, transpose=True, single_packet=False)
                    nc.gpsimd.dma_gather(
                        as3d(gsrc[:, wl:W]), table.ap()[SPLIT:N],
                        idx_src[:, slot_off + wl // 16:slot_off + W // 16],
                        wh, wh, D, transpose=True, single_packet=False)
                    nc.gpsimd.dma_gather(
                        as3d(gtgt[:, 0:W]), loc_tab.ap(),
                        idx_tgt[:, slot_off:slot_off + W // 16],
                        W, W, D, transpose=True, single_packet=False)
                    slot_off += W // 16

                    aggp = ppC.tile([128, 128], dt.float32, tag="aggp")
                    first_tile = 0
                    for g0 in range(0, W, 1024):
                        w = min(1024, W - g0)
                        nt = w // 128
                        halves = [(h0, min(512, w - h0))
                                  for h0 in range(0, w, 512)]
                        m1s, m2s = [], []
                        for ci in range(2):
                            cs = slice(ci * 128, (ci + 1) * 128)
                            p = ppA.tile([128, 1024], dt.float32, tag=f"mp{ci}")
                            for h0, hw in halves:
                                nc.tensor.matmul(
                                    p[:, h0:h0 + hw], lhsT=w1t[l][:, cs],
                                    rhs=gtgt[:, g0 + h0:g0 + h0 + hw],
                                    start=True, stop=False)
                            for h0, hw in halves:
                                nc.tensor.matmul(
                                    p[:, h0:h0 + hw], lhsT=w1s[l][:, cs],
                                    rhs=gsrc[:, g0 + h0:g0 + h0 + hw],
                                    start=False, stop=True)
                            s = mpool.tile([128, 1024], dt.bfloat16,
                                           tag=f"m1s{ci}")
                            nc.scalar.activation(s[:, :w], p[:, :w], AF.Tanh,
                                                 bias=b1[l][:, ci:ci + 1])
                            m1s.append(s)
                        for ci in range(2):
                            cs = slice(ci * 128, (ci + 1) * 128)
                            p = ppA.tile([128, 1024], dt.float32, tag=f"mp{ci}")
                            for h0, hw in halves:
                                nc.tensor.matmul(
                                    p[:, h0:h0 + hw], lhsT=w2a[l][:, cs],
                                    rhs=m1s[0][:, h0:h0 + hw],
                                    start=True, stop=False)
                            for h0, hw in halves:
                                nc.tensor.matmul(
                                    p[:, h0:h0 + hw], lhsT=w2b[l][:, cs],
                                    rhs=m1s[1][:, h0:h0 + hw],
                                    start=False, stop=True)
                            s = mpool.tile([128, 1024], dt.bfloat16,
                                           tag=f"m2s{ci}")
                            nc.scalar.activation(s[:, :w], p[:, :w], AF.Tanh,
                                                 bias=b2[l][:, ci:ci + 1])
                            m2s.append(s)
                        m3p = ppB.tile([128, 1024], dt.float32, tag="m3p")
                        for t in range(nt):
                            ts = slice(t * 128, (t + 1) * 128)
                            nc.tensor.matmul(m3p[:, ts], lhsT=m2s[0][:, ts],
                                             rhs=w3a[l][:], start=True, stop=False)
                            nc.tensor.matmul(m3p[:, ts], lhsT=m2s[1][:, ts],
                                             rhs=w3b[l][:], start=False, stop=True)
                        m3s = mpool.tile([128, 1024], dt.bfloat16, tag="m3s")
                        nc.vector.tensor_copy(m3s[:, :w], m3p[:, :w])
                        psel = mpool.tile([128, 1024], dt.bfloat16, tag="psel")
                        for t in range(nt):
                            col = tile_idx + first_tile + t
                            nc.vector.tensor_scalar(
                                psel[:, t * 128:(t + 1) * 128], iota[:],
                                tgt_rel[:, col:col + 1], None, OP.is_equal)
                        for t in range(nt):
                            ts = slice(t * 128, (t + 1) * 128)
                            nc.tensor.matmul(
                                aggp[:], lhsT=m3s[:, ts], rhs=psel[:, ts],
                                start=(first_tile + t == 0),
                                stop=(first_tile + t == W // 128 - 1))
                        first_tile += nt
                    tile_idx += W // 128
                    # agg holds unscaled segment sums; scale columns by
                    # the pre-broadcast 1/count table.
                    nc.vector.tensor_tensor(
                        agg_bf[:, b * 128:(b + 1) * 128], aggp[:],
                        cinv_bc[:, b * 128:(b + 1) * 128], OP.mult)

                # ---- GRU update over the node shard ----
                for n0, w in groups512(SHP):
                    ns = slice(n0, n0 + w)
                    rz = []
                    for k in range(2):
                        ks = slice(k * 128, (k + 1) * 128)
                        p = ppA.tile([128, 512], dt.float32, tag=f"mp{k}")
                        nc.tensor.matmul(p[:, :w], lhsT=wih[l][:, ks],
                                         rhs=agg_bf[:, ns], start=True, stop=False)
                        nc.tensor.matmul(p[:, :w], lhsT=whh[l][:, ks],
                                         rhs=h_bf[:, ns], start=False, stop=True)
                        s = mpool.tile([128, 512], dt.bfloat16, tag=f"m1s{k}")
                        nc.scalar.activation(s[:, :w], p[:, :w], AF.Sigmoid,
                                             bias=brz[l][:, k:k + 1])
                        rz.append(s)
                    gin = ppB.tile([128, 512], dt.float32, tag="m3p")
                    nc.tensor.matmul(gin[:, :w], lhsT=wih[l][:, 2 * 128:],
                                     rhs=agg_bf[:, ns], start=True, stop=True)
                    ghn = ppA.tile([128, 512], dt.float32, tag="mp0")
                    nc.tensor.matmul(ghn[:, :w], lhsT=whh[l][:, 2 * 128:],
                                     rhs=h_bf[:, ns], start=True, stop=True)
                    ghnb = mpool.tile([128, 512], dt.float32, tag="m2s0")
                    nc.vector.tensor_scalar(ghnb[:, :w], ghn[:, :w],
                                            bghn[l][:, 0:1], None, OP.add)
                    t1 = mpool.tile([128, 512], dt.float32, tag="m2s1")
                    nc.vector.tensor_tensor(t1[:, :w], rz[0][:, :w], ghnb[:, :w],
                                            OP.mult)
                    pre = mpool.tile([128, 512], dt.float32, tag="m3s")
                    nc.vector.tensor_tensor(pre[:, :w], gin[:, :w], t1[:, :w],
                                            OP.add)
                    nn = mpool.tile([128, 512], dt.float32, tag="psel")
                    nc.scalar.activation(nn[:, :w], pre[:, :w], AF.Tanh,
                                         bias=bgin[l][:, 0:1])
                    dd = mpool.tile([128, 512], dt.float32, tag="dd")
                    nc.vector.tensor_tensor(dd[:, :w], h_f32[:, ns], nn[:, :w],
                                            OP.subtract)
                    ee = mpool.tile([128, 512], dt.float32, tag="ee")
                    nc.vector.tensor_tensor(ee[:, :w], rz[1][:, :w], dd[:, :w],
                                            OP.mult)
                    nc.vector.tensor_tensor(h_f32[:, ns], nn[:, :w], ee[:, :w],
                                            OP.add)
                    nc.vector.tensor_copy(h_bf[:, ns], h_f32[:, ns])

                if l < LAYERS - 1:
                    finish_layer()

            # ---------------- decoder ----------------
            for n0, w in groups512(SHP):
                ns = slice(n0, n0 + w)
                o1s, o2s = [], []
                for ci in range(2):
                    cs = slice(ci * 128, (ci + 1) * 128)
                    p = ppA.tile([128, 512], dt.float32, tag=f"mp{ci}")
                    nc.tensor.matmul(p[:, :w], lhsT=wd1[:, cs],
                                     rhs=h_bf[:, ns], start=True, stop=True)
                    s = mpool.tile([128, 512], dt.bfloat16, tag=f"m1s{ci}")
                    nc.scalar.activation(s[:, :w], p[:, :w], AF.Tanh,
                                         bias=bd1[:, ci:ci + 1])
                    o1s.append(s)
                for ci in range(2):
                    cs = slice(ci * 128, (ci + 1) * 128)
                    p = ppA.tile([128, 512], dt.float32, tag=f"mp{ci}")
                    nc.tensor.matmul(p[:, :w], lhsT=wd2a[:, cs],
                                     rhs=o1s[0][:, :w], start=True, stop=False)
                    nc.tensor.matmul(p[:, :w], lhsT=wd2b[:, cs],
                                     rhs=o1s[1][:, :w], start=False, stop=True)
                    s = mpool.tile([128, 512], dt.bfloat16, tag=f"m2s{ci}")
                    nc.scalar.activation(s[:, :w], p[:, :w], AF.Tanh,
                                         bias=bd2[:, ci:ci + 1])
                    o2s.append(s)
                o3p = ppB.tile([1, 512], dt.float32, tag="m3p")
                nc.tensor.matmul(o3p[:, :w], lhsT=wd3a[:], rhs=o2s[0][:, :w],
                                 start=True, stop=False)
                nc.tensor.matmul(o3p[:, :w], lhsT=wd3b[:], rhs=o2s[1][:, :w],
                                 start=False, stop=True)
                yt = mpool.tile([1, 512], dt.float32, tag="m3s")
                nc.scalar.copy(yt[:, :w], o3p[:, :w])
                we = min(w, SH - n0) if n0 < SH else 0
                if we > 0:
                    nc.sync.dma_start(out=y_d.ap()[:, n0:n0 + we],
                                      in_=yt[:, :we])

    nc.compile()
    return nc


# ----------------------------------------------------------------------------
# Entry point
# ----------------------------------------------------------------------------

def make_in_maps(per_core, w):
    in_maps = []
    for c in range(NCORES):
        m = dict(per_core[c])
        m["wbf_sh"] = np.ascontiguousarray(w["wbf"][c * _WBF_SH:(c + 1) * _WBF_SH])
        m["bf32"] = w["bf32"]
        in_maps.append(m)
    return in_maps


def kernel(**inputs) -> np.ndarray:
    from concourse.bass_utils import run_bass_kernel_spmd

    meta, per_core = _prep(np.asarray(inputs["x"], np.float32),
                           np.asarray(inputs["edge_index"]))
    w = _prep_weights(inputs)

    key = (tuple(meta["Bl"]), tuple(meta["Bh"]))
    if key not in _PROGRAM_CACHE:
        _PROGRAM_CACHE[key] = _build_program(meta)
    nc = _PROGRAM_CACHE[key]

    in_maps = make_in_maps(per_core, w)
    res = run_bass_kernel_spmd(nc, in_maps, core_ids=list(range(NCORES)),
                               **_RUN_KWARGS)
    global _LAST_RESULTS
    _LAST_RESULTS = res
    out = np.concatenate([res.results[c]["y"][0] for c in range(NCORES)])
    return (out + np.asarray(inputs["dec_b3"], np.float32)[0]).astype(np.float32)
